# revision 7
# baseline (speedup 1.0000x reference)
"""Trainium2 Bass kernel for nn_CapsuleModel2 (capsule routing head).

Strategy (data-parallel, one image per NeuronCore, 8 cores):

Host-side algebraic folding:
  The whole per-pixel chain  1x1conv(poses) -> per-capsule vote conv ->
  positional-encoding linear  collapses into a single effective matmul:
     tokens_grid[(n,v), s] = Weff @ feat + (r(s)*w_d + b_eff)
  where Weff = W16 @ w_vote[n] @ w_poses[n]  (host-precomputed, 128x1280)
  and the positional encoding is rank-1 in the *grid position only*:
  pe = [(y-x)/128, (x-y)/128] so pe @ w_pos[:,16:18].T = r(s) * (wy-wx).
  That grid-constant [128,4096] table ships from the host.

Device pipeline per core (one image):
  1. tokens_grid = WeffT.T @ feat (bf16 matmul, fp32 psum) + PEGRID   [128,4096]
     z_grid = w_acts @ feat + b_acts (activation logits)              [8,4096]
  2. ap_gather (GPSIMD ucode) pulls the I*P=4096 point columns:
     tok_all[(n,v), (i,p)], zg[(n), (i,p)]
  3. Routing 1 via block-diagonal matmuls, 128-pt chunks:
     LT[pt,(n,o)] = tok_chunk.T @ blockdiag(Q1/4)        E = exp(LT)
     vals[pt,(n,j)] = (sigmoid(z)+1e-6) * (tok_chunk.T @ blockdiag(Wv1) | 1)
     numer/denom accumulate per instance: psum[j(17), o(64)] += vals_n.T @ E_n
  4. Routing 2 per instance (tiny): transpose, normalize, Q2 attention,
     class sigmoid -> out[16,19] per image.
"""

import sys

for _p in ("/opt/trn_rl_repo",):
    if _p not in sys.path:
        sys.path.insert(0, _p)

import numpy as np
import ml_dtypes

import concourse.bacc as bacc
import concourse.tile as tile
from concourse import mybir
from concourse import bass_utils

AF = mybir.ActivationFunctionType
ALU = mybir.AluOpType
F32 = mybir.dt.float32
BF16 = mybir.dt.bfloat16
I16 = mybir.dt.int16
BF16_NP = ml_dtypes.bfloat16

B, I, P = 8, 16, 256
CIN = 1280
NCAPS, DCAP, DV = 8, 32, 16
HF = WF = 64
S = HF * WF              # 4096 grid positions
NPTS = I * P             # 4096 gathered points
NOUT1, NCLS = 64, 19
KT = CIN // 128          # 10 contraction tiles
HALF = S // 2
ZW = S + 8               # z grid padded with a -inf slot for masked points
NCH = 32                 # routing-1 chunks of 128 points

_CACHE = {}


def _build_nc():
    nc = bacc.Bacc("TRN2", target_bir_lowering=False, debug=False, num_devices=8)

    din = {}

    def dram_in(name, shape, dt):
        din[name] = nc.dram_tensor(name, list(shape), dt, kind="ExternalInput").ap()
        return din[name]

    feat = dram_in("feat", (CIN, S), BF16)
    pegrid = dram_in("pegrid", (128, S), F32)
    weffT = dram_in("weffT", (CIN, 128), BF16)
    waT = dram_in("waT", (CIN, 8), BF16)
    bacts = dram_in("bacts", (8, 1), F32)
    bq1 = dram_in("bq1", (128, 512), F32)
    bwv1 = dram_in("bwv1", (128, 136), F32)
    exp8 = dram_in("exp8", (8, 136), F32)
    q2sT = dram_in("q2sT", (16, NCLS), F32)
    wact1 = dram_in("wact1", (16, 1), F32)
    ones19 = dram_in("ones19", (1, NCLS), F32)
    ones64 = dram_in("ones64", (64, 1), F32)
    wact2rep = dram_in("wact2rep", (NCLS, 16), F32)
    bact1 = dram_in("bact1", (1, 1), F32)
    eps1 = dram_in("eps1", (1, 1), F32)
    bact2rep = dram_in("bact2rep", (NCLS, 1), F32)
    ident = dram_in("ident", (128, 128), F32)
    gidx = dram_in("gidx", (128, NPTS // 16), I16)
    aidx = dram_in("aidx", (16, NPTS // 16), I16)

    out_cls = nc.dram_tensor("out_cls", [I, NCLS], F32, kind="ExternalOutput").ap()

    with tile.TileContext(nc) as tc:
        with (
            tc.tile_pool(name="cons", bufs=1) as cons,
            tc.tile_pool(name="grid", bufs=1) as grid,
            tc.tile_pool(name="feats", bufs=3) as feats,
            tc.tile_pool(name="rsb", bufs=3) as rsb,
            tc.tile_pool(name="small", bufs=2) as small,
        ):
            # ---- constants to SBUF ----
            pegrid_sb = cons.tile([128, S], F32)
            nc.sync.dma_start(out=pegrid_sb[:], in_=pegrid)
            weffT_sb = cons.tile([128, KT, 128], BF16)
            nc.sync.dma_start(
                out=weffT_sb[:], in_=weffT.rearrange("(k p) m -> p k m", p=128)
            )
            waT_sb = cons.tile([128, KT, 8], BF16)
            nc.sync.dma_start(
                out=waT_sb[:], in_=waT.rearrange("(k p) m -> p k m", p=128)
            )
            bacts_sb = cons.tile([8, 1], F32)
            nc.sync.dma_start(out=bacts_sb[:], in_=bacts)
            bq1_sb = cons.tile([128, 512], F32)
            nc.sync.dma_start(out=bq1_sb[:], in_=bq1)
            bwv1_sb = cons.tile([128, 136], F32)
            nc.sync.dma_start(out=bwv1_sb[:], in_=bwv1)
            exp8_sb = cons.tile([8, 136], F32)
            nc.sync.dma_start(out=exp8_sb[:], in_=exp8)
            q2sT_sb = cons.tile([16, NCLS], F32)
            nc.sync.dma_start(out=q2sT_sb[:], in_=q2sT)
            wact1_sb = cons.tile([16, 1], F32)
            nc.sync.dma_start(out=wact1_sb[:], in_=wact1)
            ones19_sb = cons.tile([1, NCLS], F32)
            nc.sync.dma_start(out=ones19_sb[:], in_=ones19)
            ones64_sb = cons.tile([64, 1], F32)
            nc.sync.dma_start(out=ones64_sb[:], in_=ones64)
            wact2rep_sb = cons.tile([NCLS, 16], F32)
            nc.sync.dma_start(out=wact2rep_sb[:], in_=wact2rep)
            bact1_sb = cons.tile([1, 1], F32)
            nc.sync.dma_start(out=bact1_sb[:], in_=bact1)
            eps1_sb = cons.tile([1, 1], F32)
            nc.sync.dma_start(out=eps1_sb[:], in_=eps1)
            bact2rep_sb = cons.tile([NCLS, 1], F32)
            nc.sync.dma_start(out=bact2rep_sb[:], in_=bact2rep)
            ident_sb = cons.tile([128, 128], F32)
            nc.sync.dma_start(out=ident_sb[:], in_=ident)
            gidx_sb = cons.tile([128, NPTS // 16], I16)
            nc.sync.dma_start(out=gidx_sb[:], in_=gidx)
            aidx_sb = cons.tile([16, NPTS // 16], I16)
            nc.sync.dma_start(out=aidx_sb[:], in_=aidx)

            # ---- persistent grid tensors ----
            tokens_sb = grid.tile([128, S], F32)
            z_sb = grid.tile([16, ZW], F32)
            tok_all = grid.tile([128, NPTS], F32)
            zg_all = grid.tile([16, NPTS], F32)
            outcls_sb = grid.tile([NCLS, I], F32)
            nc.vector.memset(z_sb[:, :], 0.0)
            nc.vector.memset(z_sb[0:8, S:ZW], -10000.0)

            # ---- phase G: grid matmuls ----
            with tc.tile_pool(name="pgrid", bufs=1, space="PSUM") as pg:
                for h in range(2):
                    pms = [
                        pg.tile([128, 512], F32, tag=f"pm{nn}", name=f"pm{nn}_{h}")
                        for nn in range(4)
                    ]
                    pas = [
                        pg.tile([8, 512], F32, tag=f"pa{nn}", name=f"pa{nn}_{h}")
                        for nn in range(4)
                    ]
                    for k in range(KT):
                        ft = feats.tile([128, HALF], BF16, tag="feat")
                        nc.sync.dma_start(
                            out=ft[:],
                            in_=feat[k * 128 : (k + 1) * 128, h * HALF : (h + 1) * HALF],
                        )
                        for nn in range(4):
                            nc.tensor.matmul(
                                pms[nn][:],
                                lhsT=weffT_sb[:, k, :],
                                rhs=ft[:, nn * 512 : (nn + 1) * 512],
                                start=(k == 0),
                                stop=(k == KT - 1),
                            )
                        for nn in range(4):
                            nc.tensor.matmul(
                                pas[nn][:],
                                lhsT=waT_sb[:, k, :],
                                rhs=ft[:, nn * 512 : (nn + 1) * 512],
                                start=(k == 0),
                                stop=(k == KT - 1),
                            )
                    for nn in range(4):
                        off = h * HALF + nn * 512
                        nc.vector.tensor_add(
                            out=tokens_sb[:, off : off + 512],
                            in0=pms[nn][:],
                            in1=pegrid_sb[:, off : off + 512],
                        )
                        nc.scalar.activation(
                            out=z_sb[0:8, off : off + 512],
                            in_=pas[nn][:],
                            func=AF.Identity,
                            bias=bacts_sb[:],
                            scale=1.0,
                        )

            # ---- phase H: gathers (GPSIMD ucode) ----
            QIDX = NPTS // 4
            for q in range(4):
                nc.gpsimd.ap_gather(
                    tok_all[:, q * QIDX : (q + 1) * QIDX],
                    tokens_sb[:],
                    gidx_sb[:, q * (QIDX // 16) : (q + 1) * (QIDX // 16)],
                    channels=128,
                    num_elems=S,
                    d=1,
                    num_idxs=QIDX,
                )
                nc.gpsimd.ap_gather(
                    zg_all[:, q * QIDX : (q + 1) * QIDX],
                    z_sb[:],
                    aidx_sb[:, q * (QIDX // 16) : (q + 1) * (QIDX // 16)],
                    channels=16,
                    num_elems=ZW,
                    d=1,
                    num_idxs=QIDX,
                )

            # ---- phase R1 + R2: routing ----
            with (
                tc.tile_pool(name="pl", bufs=2, space="PSUM") as plp,
                tc.tile_pool(name="pv", bufs=2, space="PSUM") as pvp,
                tc.tile_pool(name="pa2", bufs=1, space="PSUM") as pap,
                tc.tile_pool(name="pn", bufs=2, space="PSUM") as pnp,
                tc.tile_pool(name="pr2", bufs=1, space="PSUM") as pr2p,
            ):
                pn = None
                for c in range(NCH):
                    inst = c // 2
                    even = c % 2 == 0
                    tokc = tok_all[:, c * 128 : (c + 1) * 128]

                    pl = plp.tile([128, 512], F32, tag="pl")
                    nc.tensor.matmul(
                        pl[:], lhsT=tokc, rhs=bq1_sb[:], start=True, stop=True
                    )
                    E = rsb.tile([128, 512], F32, tag="E")
                    nc.scalar.activation(out=E[:], in_=pl[:], func=AF.Exp)

                    pv = pvp.tile([128, 136], F32, tag="pv")
                    nc.tensor.matmul(
                        pv[:], lhsT=tokc, rhs=bwv1_sb[:], start=True, stop=True
                    )
                    pa2 = pap.tile([128, 136], F32, tag="pa2")
                    nc.tensor.matmul(
                        pa2[:],
                        lhsT=zg_all[0:8, c * 128 : (c + 1) * 128],
                        rhs=exp8_sb[:],
                        start=True,
                        stop=True,
                    )
                    asig = rsb.tile([128, 136], F32, tag="asig")
                    nc.scalar.activation(out=asig[:], in_=pa2[:], func=AF.Sigmoid)

                    vals = rsb.tile([128, 136], F32, tag="vals")
                    nc.vector.scalar_tensor_tensor(
                        out=vals[:],
                        in0=asig[:],
                        scalar=1e-6,
                        in1=pv[:],
                        op0=ALU.add,
                        op1=ALU.mult,
                    )
                    vr = vals[:].rearrange("p (n j) -> p n j", j=17)
                    ar = asig[:].rearrange("p (n j) -> p n j", j=17)
                    nc.vector.tensor_scalar_add(
                        out=vr[:, :, 16:17], in0=ar[:, :, 16:17], scalar1=1e-6
                    )

                    if even:
                        pn = pnp.tile([17, 64], F32, tag="pn")
                    for n in range(8):
                        nc.tensor.matmul(
                            pn[:],
                            lhsT=vals[:, n * 17 : (n + 1) * 17],
                            rhs=E[:, n * 64 : (n + 1) * 64],
                            start=(even and n == 0),
                            stop=((not even) and n == 7),
                            skip_group_check=True,
                        )

                    if not even:
                        # ---- routing 2 for this instance ----
                        acc_sb = small.tile([17, 64], F32, tag="acc")
                        nc.vector.tensor_copy(out=acc_sb[:], in_=pn[:])
                        pT = pr2p.tile([64, 17], F32, tag="r2")
                        nc.tensor.transpose(
                            out=pT[:], in_=acc_sb[:], identity=ident_sb[0:17, 0:17]
                        )
                        recd = small.tile([64, 1], F32, tag="recd")
                        nc.vector.reciprocal(out=recd[:], in_=pT[:, 16:17])
                        p1i = small.tile([64, 16], F32, tag="p1i")
                        nc.vector.tensor_scalar_mul(
                            out=p1i[:], in0=pT[:, 0:16], scalar1=recd[:]
                        )
                        pP = pr2p.tile([16, 64], F32, tag="r2")
                        nc.tensor.transpose(
                            out=pP[:], in_=p1i[:], identity=ident_sb[0:64, 0:64]
                        )
                        p1T = small.tile([16, 64], F32, tag="p1T")
                        nc.vector.tensor_copy(out=p1T[:], in_=pP[:])

                        pa1 = pr2p.tile([1, 64], F32, tag="r2")
                        nc.tensor.matmul(
                            pa1[:], lhsT=wact1_sb[:], rhs=p1T[:], start=True, stop=True
                        )
                        s1 = small.tile([1, 64], F32, tag="s1")
                        nc.scalar.activation(
                            out=s1[:], in_=pa1[:], func=AF.Sigmoid, bias=bact1_sb[:]
                        )
                        la1 = small.tile([1, 64], F32, tag="la1")
                        nc.scalar.activation(
                            out=la1[:], in_=s1[:], func=AF.Ln, bias=eps1_sb[:]
                        )

                        pL2 = pr2p.tile([64, NCLS], F32, tag="r2")
                        nc.tensor.matmul(
                            pL2[:],
                            lhsT=p1T[:],
                            rhs=q2sT_sb[:],
                            start=True,
                            stop=False,
                            skip_group_check=True,
                        )
                        nc.tensor.matmul(
                            pL2[:],
                            lhsT=la1[:],
                            rhs=ones19_sb[:],
                            start=False,
                            stop=True,
                            skip_group_check=True,
                        )
                        E2 = small.tile([64, NCLS], F32, tag="E2")
                        nc.scalar.activation(out=E2[:], in_=pL2[:], func=AF.Exp)

                        pnd = pr2p.tile([NCLS, 17], F32, tag="r2")
                        nc.tensor.matmul(
                            pnd[:, 0:16],
                            lhsT=E2[:],
                            rhs=p1i[:],
                            start=True,
                            stop=True,
                            skip_group_check=True,
                        )
                        nc.tensor.matmul(
                            pnd[:, 16:17],
                            lhsT=E2[:],
                            rhs=ones64_sb[:],
                            start=True,
                            stop=True,
                            skip_group_check=True,
                        )
                        recd2 = small.tile([NCLS, 1], F32, tag="recd2")
                        nc.vector.reciprocal(out=recd2[:], in_=pnd[:, 16:17])
                        p2 = small.tile([NCLS, 16], F32, tag="p2")
                        nc.vector.tensor_scalar_mul(
                            out=p2[:], in0=pnd[:, 0:16], scalar1=recd2[:]
                        )
                        zt = small.tile([NCLS, 16], F32, tag="zt")
                        nc.vector.tensor_mul(out=zt[:], in0=p2[:], in1=wact2rep_sb[:])
                        z2 = small.tile([NCLS, 1], F32, tag="z2")
                        nc.vector.reduce_sum(
                            out=z2[:], in_=zt[:], axis=mybir.AxisListType.X
                        )
                        nc.scalar.activation(
                            out=outcls_sb[:, inst : inst + 1],
                            in_=z2[:],
                            func=AF.Sigmoid,
                            bias=bact2rep_sb[:],
                        )

            nc.sync.dma_start(
                out=out_cls.rearrange("i c -> c i"), in_=outcls_sb[:]
            )

    nc.compile()
    return nc


def _get_nc():
    if "nc" not in _CACHE:
        _CACHE["nc"] = _build_nc()
    return _CACHE["nc"]


def _wrap_idx(sidx):
    # ap_gather index layout: index j lives at partition j%16, column j//16.
    return np.ascontiguousarray(sidx.reshape(-1, 16).T.astype(np.int16))


def host_prep(inputs):
    """Build the per-core input maps (all numpy, host-side weight folding)."""
    f8 = np.float64
    w_pos = np.asarray(inputs["w_pos"], f8)          # (16, 18)
    W16 = w_pos[:, :16]
    w_d = w_pos[:, 16] - w_pos[:, 17]                # (16,)
    b_pos = np.asarray(inputs["b_pos"], f8)
    w_vote = np.asarray(inputs["w_vote"], f8)        # (8, 16, 32)
    b_vote = np.asarray(inputs["b_vote"], f8)        # (8, 16)
    Wp = np.asarray(inputs["w_poses"], f8).reshape(NCAPS, DCAP, CIN)
    b_poses = np.asarray(inputs["b_poses"], f8).reshape(NCAPS, DCAP)

    Weff = np.stack([W16 @ w_vote[n] @ Wp[n] for n in range(NCAPS)])  # (8,16,1280)
    beff = np.stack(
        [W16 @ (w_vote[n] @ b_poses[n] + b_vote[n]) + b_pos for n in range(NCAPS)]
    )                                                                  # (8,16)
    Weff = Weff.reshape(128, CIN)
    beff = beff.reshape(128)

    ss = np.arange(S)
    r = ((ss // WF) - (ss % WF)) / 128.0
    pegrid = (np.tile(w_d, NCAPS)[:, None] * r[None, :] + beff[:, None]).astype(
        np.float32
    )

    Q1s = np.asarray(inputs["Q1"], f8) / 4.0         # (64, 16)
    BQ1 = np.zeros((128, 512), np.float32)
    for n in range(NCAPS):
        BQ1[n * 16 : (n + 1) * 16, n * 64 : (n + 1) * 64] = Q1s.T
    Wv1 = np.asarray(inputs["Wv1"], f8)
    BWV1 = np.zeros((128, 136), np.float32)
    for n in range(NCAPS):
        BWV1[n * 16 : (n + 1) * 16, n * 17 : n * 17 + 16] = Wv1
    EXP8 = np.zeros((8, 136), np.float32)
    for n in range(NCAPS):
        EXP8[n, n * 17 : (n + 1) * 17] = 1.0

    consts = dict(
        pegrid=pegrid,
        weffT=np.ascontiguousarray(Weff.T).astype(BF16_NP),
        waT=np.ascontiguousarray(np.asarray(inputs["w_acts"], f8).T).astype(BF16_NP),
        bacts=np.asarray(inputs["b_acts"], np.float32).reshape(8, 1),
        bq1=BQ1,
        bwv1=BWV1,
        exp8=EXP8,
        q2sT=np.ascontiguousarray((np.asarray(inputs["Q2"], f8) / 4.0).T).astype(
            np.float32
        ),
        wact1=np.asarray(inputs["wact1"], np.float32).reshape(16, 1),
        ones19=np.ones((1, NCLS), np.float32),
        ones64=np.ones((64, 1), np.float32),
        wact2rep=np.tile(
            np.asarray(inputs["wact2"], np.float32).reshape(1, 16), (NCLS, 1)
        ),
        bact1=np.asarray(inputs["bact1"], np.float32).reshape(1, 1),
        eps1=np.full((1, 1), 1e-6, np.float32),
        bact2rep=np.full((NCLS, 1), float(np.asarray(inputs["bact2"])), np.float32),
        ident=np.eye(128, dtype=np.float32),
    )

    feats = np.asarray(inputs["feature_output"])     # (8, 1280, 64, 64) f32
    coords = np.asarray(inputs["point_coords"])      # (8, 16, 2, 256) int32
    mask = np.asarray(inputs["point_mask"])          # (8, 16, 256) bool

    in_maps = []
    for b in range(B):
        y = np.clip(coords[b, :, 0, :], 0, HF - 1).astype(np.int64)
        x = np.clip(coords[b, :, 1, :], 0, WF - 1).astype(np.int64)
        sidx = (y * WF + x).reshape(NPTS)
        zidx = sidx.copy()
        mb = mask[b].reshape(NPTS)
        zidx[~mb] = S  # masked points read the -1e4 z slot -> ~zero weight
        m = dict(consts)
        m["feat"] = np.ascontiguousarray(
            feats[b].reshape(CIN, S).astype(BF16_NP)
        )
        m["gidx"] = np.tile(_wrap_idx(sidx), (8, 1))
        m["aidx"] = _wrap_idx(zidx)
        in_maps.append(m)
    return in_maps


def kernel(**inputs):
    nc = _get_nc()
    in_maps = host_prep(inputs)
    res = bass_utils.run_bass_kernel_spmd(nc, in_maps, core_ids=list(range(B)))
    out = np.stack([np.asarray(res.results[b]["out_cls"]) for b in range(B)])
    return out.astype(np.float32)
